# revision 10
# baseline (speedup 1.0000x reference)
"""Trainium2 Bass kernel for batched weighted complex Gram matrices.

Reference (per batch b, R/I = input_real/imag[b] (S=1024, D=256), w>=0):
    out_r = R^T diag(w) R + I^T diag(w) I      (symmetric)
    out_i = I^T diag(w) R - R^T diag(w) I      (antisymmetric)

Key algebra: with M = [R | I] (S x 2D) and m = diag(sqrt(w)) M, the Gram
G = m^T m (512x512, symmetric) contains everything:
    G = [[P, Y],[Y^T, Q]],  P = R^T W R, Q = I^T W I, Y = R^T W I
    out_r = P + Q            out_i = Y^T - Y
Per 128-chunk of the contraction, 4 PE matmuls (moving widths
512/256/384/256) accumulate: pA = [P00+Q00, P01+Q01, Y00, Y01]
(Q-blocks PE-accumulated in place), pB = [P11, Y10, Y11],
pCD = [Q10, Q11]. 1408 moving rows per chunk vs 2048 naive (-31%).

sqrt(w) is folded into the tiny [128, 32] weight tensor on the host;
on-device prep is ONE 512-col scaled copy per chunk (f32 -> f32r),
alternating between the ACT and DVE engines.

Pipeline (batch-granular software pipeline over in-order engine queues;
epilogue of batch b is spread over iterations b+1 / b+2 so it never
blocks preps, and PE transposes of batch b-1 run right after batch b's
matmuls to keep the PE saturated at its ramped clock):
    iter b: preps(b) | precopy(b-1) | mms(b), T(b-1) | finish(b-2)

Sharding: data-parallel over batch, 4 batches per core x 8 cores.
Layout: s = p*8 + c so every DMA descriptor is a contiguous 8KB run per
partition. All input DMAs are issued up-front on the sync ring; outputs
(fp16, host casts to f32) leave per batch.
"""

import sys

if "/opt/trn_rl_repo" not in sys.path:
    sys.path.insert(0, "/opt/trn_rl_repo")

import numpy as np

B, S, D = 32, 1024, 256
NCORES = 8
NB = B // NCORES          # batches per core
NCH = S // 128            # contraction chunks per batch

# tunables
WARMUP_MMS = 5            # dummy matmuls to pre-warm the PE p-state
B0_PIECES = [(0, 2), (2, 8)]  # batch-0 input DMA pieces (chunk ranges)
OUT_RING = "gpsimd"       # engine ring for output DMAs

_compiled = {}


def _build():
    import concourse.bacc as bacc
    import concourse.tile as tile
    import concourse.mybir as mybir

    f32 = mybir.dt.float32
    f32r = mybir.dt.float32r
    f16 = mybir.dt.float16

    nc = bacc.Bacc("TRN2", target_bir_lowering=False, debug=False)
    r_d = nc.dram_tensor("r", [NB, S, D], f32, kind="ExternalInput")
    i_d = nc.dram_tensor("i", [NB, S, D], f32, kind="ExternalInput")
    # host-pretransposed sqrt-weights: wsq[p, b*NCH+c] = sqrt(w[b, p*NCH+c])
    wsq_d = nc.dram_tensor("wsq", [128, NB * NCH], f32, kind="ExternalInput")
    # outputs as [b, p, a, d]; host maps (a, p) -> row a*128+p and casts to f32
    or_d = nc.dram_tensor("o_r", [NB, 128, 2, 256], f16, kind="ExternalOutput")
    oi_d = nc.dram_tensor("o_i", [NB, 128, 2, 256], f16, kind="ExternalOutput")

    out_dma = {
        "scalar": lambda *a: nc.scalar.dma_start(*a),
        "vector": lambda *a: nc.vector.dma_start(*a),
        "gpsimd": lambda *a: nc.gpsimd.dma_start(*a),
        "sync": lambda *a: nc.sync.dma_start(*a),
    }[OUT_RING]

    with tile.TileContext(nc) as tc:
        with (
            tc.tile_pool(name="wpool", bufs=1) as wpool,
            tc.tile_pool(name="xp", bufs=4) as xp,
            tc.tile_pool(name="mp", bufs=2) as mp,
            tc.tile_pool(name="yp", bufs=3) as yp,
            tc.tile_pool(name="op", bufs=3) as op,
            tc.tile_pool(name="ps", bufs=2, space="PSUM") as ps,
        ):
            w_sc = wpool.tile([128, NB * NCH], f32)
            warm = wpool.tile([128, 1], f32)
            ident = wpool.tile([128, 128], f16)
            nc.vector.memset(warm[:], 0.0)
            nc.scalar.copy(warm[:], warm[:])  # prime ACT table load early
            nc.sync.dma_start(w_sc[:], wsq_d[:])

            ident32 = wpool.tile([128, 128], f32)
            nc.vector.memset(ident32[:], 1.0)
            nc.gpsimd.affine_select(
                out=ident32[:],
                in_=ident32[:],
                compare_op=mybir.AluOpType.is_equal,
                fill=0.0,
                base=0,
                pattern=[[-1, 128]],
                channel_multiplier=1,
            )
            nc.scalar.copy(ident[:], ident32[:])

            if WARMUP_MMS:
                wz = wpool.tile([128, 512], f32)
                nc.vector.memset(wz[:], 0.0)
                pwarm = ps.tile([128, 512], f32, name="pwarm", bufs=1)
                for _ in range(WARMUP_MMS):
                    nc.tensor.matmul(
                        pwarm[:],
                        wz[:, 0:128].bitcast(f32r),
                        wz[:].bitcast(f32r),
                        start=True, stop=True, skip_group_check=True,
                    )

            # s = p*NCH + c  =>  per-partition contiguous rows in DRAM
            ir_re = i_d.rearrange("b (p c) d -> b p c d", p=128)
            rr_re = r_d.rearrange("b (p c) d -> b p c d", p=128)

            # --- issue ALL input DMAs up-front on the sync ring ---
            # x[b][:, 0] = R chunks, x[b][:, 1] = I chunks (8KB runs each)
            x = [xp.tile([128, 2, NCH, 256], f32, name="x") for _ in range(NB)]
            for c0, c1 in B0_PIECES:
                nc.sync.dma_start(x[0][:, 0, c0:c1, :], rr_re[0, :, c0:c1, :])
                nc.sync.dma_start(x[0][:, 1, c0:c1, :], ir_re[0, :, c0:c1, :])
            for b in range(1, NB):
                nc.sync.dma_start(x[b][:, 0], rr_re[b])
                nc.sync.dma_start(x[b][:, 1], ir_re[b])

            state = {}

            def emit_preps(b):
                wm = mp.tile([128, NCH, 2, 256], f32r, name="wm")
                state[b] = {"wm": wm}
                for c in range(NCH):
                    col = b * NCH + c
                    src = x[b][:, :, c, :]
                    dst = wm[:, c, :, :]
                    if c % 2 == 0:
                        nc.vector.tensor_scalar_mul(dst, src, w_sc[:, col:col + 1])
                    else:
                        nc.scalar.mul(dst, src, w_sc[:, col:col + 1])

            def emit_mms(b):
                st = state[b]
                wm = st["wm"]
                wmf = wm[:].rearrange("p c t d -> p c (t d)")
                st["ps"] = (
                    ps.tile([128, 512], f32, name="pA"),
                    ps.tile([128, 384], f32, name="pB"),
                    ps.tile([128, 256], f32, name="pCD"),
                )
                pA, pB, pCD = st["ps"]
                for c in range(NCH):
                    s0, sp = (c == 0), (c == NCH - 1)
                    nc.tensor.matmul(
                        pA[:], wmf[:, c, 0:128], wmf[:, c, 0:512],
                        start=s0, stop=False, skip_group_check=True,
                    )
                    nc.tensor.matmul(
                        pA[:, 0:256], wmf[:, c, 256:384], wmf[:, c, 256:512],
                        start=False, stop=sp, skip_group_check=True,
                    )
                    nc.tensor.matmul(
                        pB[:], wmf[:, c, 128:256], wmf[:, c, 128:512],
                        start=s0, stop=sp, skip_group_check=True,
                    )
                    nc.tensor.matmul(
                        pCD[:], wmf[:, c, 384:512], wmf[:, c, 256:512],
                        start=s0, stop=sp, skip_group_check=True,
                    )

            def emit_precopy(b):
                st = state[b]
                pA, pB, pCD = st["ps"]
                y_sb = st["y"] = yp.tile([128, 4, 128], f16, name="y_sb")
                q_sb = st["q"] = yp.tile([128, 128], f16, name="q_sb")
                or_sb = st["or"] = op.tile([128, 2, 256], f16, name="or_sb")
                st["oi"] = op.tile([128, 2, 256], f16, name="oi_sb")
                # out_r row-block 0 is ready in PSUM: [r00 r01]
                nc.scalar.copy(or_sb[:, 0, :], pA[:, 0:256])
                nc.scalar.copy(q_sb[:], pCD[:, 128:256])
                nc.vector.tensor_copy(y_sb[:, 0, :], pA[:, 256:384])   # Y00
                nc.vector.tensor_copy(y_sb[:, 1, :], pA[:, 384:512])   # Y01
                nc.vector.tensor_copy(y_sb[:, 2, :], pB[:, 128:256])   # Y10
                nc.vector.tensor_copy(y_sb[:, 3, :], pB[:, 256:384])   # Y11
                # r11 = P11 + Q11
                nc.vector.tensor_add(or_sb[:, 1, 128:256], pB[:, 0:128], q_sb[:])

            def emit_T(b):
                st = state[b]
                y_sb, or_sb = st["y"], st["or"]
                pT = st["pT"] = ps.tile([128, 640], f16, name="pT", bufs=1)
                nc.tensor.transpose(pT[:, 0:128], y_sb[:, 0, :], ident[:])
                nc.tensor.transpose(pT[:, 128:256], y_sb[:, 1, :], ident[:])
                nc.tensor.transpose(pT[:, 256:384], y_sb[:, 2, :], ident[:])
                nc.tensor.transpose(pT[:, 384:512], y_sb[:, 3, :], ident[:])
                nc.tensor.transpose(pT[:, 512:640], or_sb[:, 0, 128:256], ident[:])

            def emit_finish(b):
                st = state[b]
                y_sb, or_sb, oi_sb, pT = st["y"], st["or"], st["oi"], st["pT"]
                # out_i = Y^T - Y
                nc.vector.tensor_sub(oi_sb[:, 0, 0:128], pT[:, 0:128], y_sb[:, 0, :])
                nc.vector.tensor_sub(oi_sb[:, 0, 128:256], pT[:, 256:384], y_sb[:, 1, :])
                nc.vector.tensor_sub(oi_sb[:, 1, 0:128], pT[:, 128:256], y_sb[:, 2, :])
                nc.vector.tensor_sub(oi_sb[:, 1, 128:256], pT[:, 384:512], y_sb[:, 3, :])
                # out_r lower-left = r01^T
                nc.scalar.copy(or_sb[:, 1, 0:128], pT[:, 512:640])
                out_dma(or_d[b], or_sb[:])
                out_dma(oi_d[b], oi_sb[:])

            for b in range(NB):
                emit_preps(b)
                if b >= 1:
                    emit_precopy(b - 1)
                emit_mms(b)
                if b >= 1:
                    emit_T(b - 1)
                if b >= 2:
                    emit_finish(b - 2)
            emit_precopy(NB - 1)
            emit_T(NB - 1)
            emit_finish(NB - 2)
            emit_finish(NB - 1)

    nc.compile()
    return nc


def _get_nc():
    if "nc" not in _compiled:
        _compiled["nc"] = _build()
    return _compiled["nc"]


def run(input_real, input_imag, weights, trace=False):
    from concourse.bass_utils import run_bass_kernel_spmd

    nc = _get_nc()
    w = np.sqrt(np.asarray(weights, dtype=np.float64)).astype(np.float32)
    in_maps = []
    for c in range(NCORES):
        sl = slice(NB * c, NB * (c + 1))
        # wsq[p, b*NCH+ch] = sqrt(w)[b, p*NCH+ch]   (s = p*NCH + ch)
        wsq = np.ascontiguousarray(
            w[sl].reshape(NB, 128, NCH).transpose(1, 0, 2).reshape(128, NB * NCH)
        )
        in_maps.append(
            {
                "r": np.ascontiguousarray(input_real[sl], dtype=np.float32),
                "i": np.ascontiguousarray(input_imag[sl], dtype=np.float32),
                "wsq": wsq,
            }
        )
    res = run_bass_kernel_spmd(
        nc, in_maps, core_ids=list(range(NCORES)), trace=trace
    )

    def assemble(key):
        # [NB, 128, 2, 256] f16 per core -> [B, 256, 256] f32
        parts = []
        for c in range(NCORES):
            arr = np.asarray(res.results[c][key])
            parts.append(
                arr.transpose(0, 2, 1, 3).reshape(NB, 256, 256).astype(np.float32)
            )
        return np.concatenate(parts, axis=0)

    return (assemble("o_r"), assemble("o_i")), res


def kernel(input_real, input_imag, weights):
    (out_r, out_i), _ = run(input_real, input_imag, weights, trace=False)
    return (out_r, out_i)
